# revision 1
# baseline (speedup 1.0000x reference)
"""Trainium2 Bass kernel for nn_CosBlock (cos-attention transformer block).

Computation (B=4, T=2048, D=1024, H=16, Dh=64, Dmlp=4096), fp32:
    y  = LN1(x)
    q,k = tanh(y @ Wq|k) * pi/4 ; V = y @ Wv          (per head)
    cos-linear-attention via causal cumsum over T, normalized
    x2 = x + attn @ Wo
    out = x2 + gelu(LN2(x2) @ W1 + b1) @ W2 + b2

Distribution: tokens sharded over 8 cores (T split into 8 chunks of 256
per batch).  The only cross-core dependency is the cumsum carry: each
core AllGathers its per-128-block partial sums (tiny, 66KB/core) and
adds a prefix offset computed with a per-core 0/1 mask matmul.

All matmuls run in float32r (TF32-like PE mode: 1 cyc/row, ~1.5e-4 rel
err).  Cumsum is an upper-triangular matmul per 128-token block with the
carry offsets folded into the same PSUM accumulation via K=1 matmuls.
"""
from contextlib import ExitStack

import numpy as np

import concourse.bacc as bacc
import concourse.tile as tile
import concourse.mybir as mybir
from concourse import bass2jax

F32 = mybir.dt.float32
F32R = mybir.dt.float32r
AF = mybir.ActivationFunctionType
PI = float(np.pi)
LN_EPS = 1e-5
COS_EPS = 1e-6

NCORES = 8
B, T, D, H, DH, DMLP = 4, 2048, 1024, 16, 64, 4096
TC = T // NCORES          # tokens per core per batch = 256
NTOK = B * TC             # tokens per core = 1024
NTT = NTOK // 128         # token tiles per core = 8  (= B * 2 blocks)
NDK = D // 128            # 8
NMT = DMLP // 128         # 32
SCOLS = 2 * H * DH + 2 * H  # 2080 = cos*V | sin*V | cos_k | sin_k


def build_program(trivial_ln=True, repeats=1, n_devices=NCORES,
                  skip_collective=False, phase_marks=None):
    nc = bacc.Bacc("TRN2", target_bir_lowering=False, debug=False,
                   num_devices=n_devices)

    def din(name, shape, dt=F32R):
        return nc.dram_tensor(name, shape, dt, kind="ExternalInput").ap()

    xs_d = din("xs", [NTOK, D])
    wqk_d = din("wqk", [D, 2 * H])
    wv_d = din("wv", [D, H * DH])
    wo_d = din("wo", [H * DH, D])
    w1_d = din("w1", [D, DMLP])
    w2_d = din("w2", [DMLP, D])
    b1_d = din("b1r", [128, NMT], F32)
    b2_d = din("b2r", [1, D])
    u_d = din("utri", [128, 128])
    eye_d = din("eye", [128, 128])
    ones128_d = din("ones128", [128, 1])
    ones1_d = din("ones1", [1, 128])
    mask_d = din("masks", [2 * NCORES, 2])
    maskrep_d = din("maskrep", [2 * NCORES, 2 * 128])
    lnw_d = din("lnw", [4, D], F32)  # ln1_w, ln1_b, ln2_w, ln2_b rows
    out_d = nc.dram_tensor("out", [NTOK, D], F32, kind="ExternalOutput").ap()

    with tile.TileContext(nc) as tc, ExitStack() as top:
        consts = top.enter_context(tc.tile_pool(name="consts", bufs=1))
        u_sb = consts.tile([128, 128], F32R)
        eye_sb = consts.tile([128, 128], F32R)
        ones128 = consts.tile([128, 1], F32R)
        ones1 = consts.tile([1, 128], F32R)
        mask_sb = consts.tile([2 * NCORES, 2], F32R)
        maskrep = consts.tile([2 * NCORES, 2 * 128], F32R)
        eps_t = consts.tile([128, 1], F32)
        halfpi = consts.tile([128, 1], F32)
        cose_t = consts.tile([128, 1], F32)
        b1_sb = consts.tile([128, NMT], F32)
        b2_sb = consts.tile([1, D], F32R)
        wqk_sb = consts.tile([128, NDK, 2 * H], F32R)
        nc.sync.dma_start(u_sb[:], u_d)
        nc.sync.dma_start(eye_sb[:], eye_d)
        nc.sync.dma_start(ones128[:], ones128_d)
        nc.sync.dma_start(ones1[:], ones1_d)
        nc.sync.dma_start(wqk_sb[:],
                          wqk_d.rearrange("(k p) n -> p k n", p=128))
        # not needed until phases D/E — keep them off the sync queue so
        # the first x-tile loads go out immediately
        nc.gpsimd.dma_start(mask_sb[:], mask_d)
        nc.gpsimd.dma_start(maskrep[:], maskrep_d)
        nc.gpsimd.dma_start(b1_sb[:], b1_d)
        nc.gpsimd.dma_start(b2_sb[:], b2_d)
        nc.vector.memset(eps_t[:], LN_EPS)
        nc.vector.memset(halfpi[:], PI / 2)
        nc.vector.memset(cose_t[:], COS_EPS)
        lnw_sb = None
        if not trivial_ln:
            lnw_sb = consts.tile([128, 4, D], F32)
            nc.sync.dma_start(
                lnw_sb[:], lnw_d[None, :, :].broadcast_to([128, 4, D]))

        for _rep in range(repeats):
            _body(nc, tc, trivial_ln, skip_collective, phase_marks,
                  xs_d, wv_d, wo_d, w1_d, w2_d, out_d,
                  u_sb, eye_sb, ones128, ones1, mask_sb, maskrep, eps_t,
                  halfpi, cose_t, b1_sb, b2_sb, wqk_sb, lnw_sb)

    nc.compile()
    return nc


def _layernorm(nc, pool, x_t, y_t, eps_t, lnw_sb, widx):
    """token-major LN: y_t[128,1024] = LN(x_t).  lnw_sb rows widx,widx+1."""
    stats = pool.tile([128, 6 * nc.vector.BN_STATS_DIM], F32, tag="ln_stats")
    nsub = D // 512
    st3 = stats[:].rearrange("p (s d) -> p s d", s=6)
    xg = x_t[:].rearrange("p (s d) -> p s d", s=nsub)
    for s in range(nsub):
        nc.vector.bn_stats(out=st3[:, s, :], in_=xg[:, s, :])
    mv = pool.tile([128, nc.vector.BN_AGGR_DIM], F32, tag="ln_mv")
    nc.vector.bn_aggr(out=mv[:], in_=stats[:, : nsub * nc.vector.BN_STATS_DIM]
                      .rearrange("p (s d) -> p s d", s=nsub))
    rstd = pool.tile([128, 1], F32, tag="ln_rstd")
    nc.scalar.activation(out=rstd[:], in_=mv[:, 1:2], func=AF.Sqrt,
                         bias=eps_t[:], scale=1.0)
    nc.vector.reciprocal(rstd[:], rstd[:])
    nc.vector.tensor_scalar(
        out=y_t[:], in0=x_t[:], scalar1=mv[:, 0:1], scalar2=rstd[:],
        op0=mybir.AluOpType.subtract, op1=mybir.AluOpType.mult)
    if lnw_sb is not None:
        nc.vector.tensor_mul(y_t[:], y_t[:], lnw_sb[:, widx, :])
        nc.vector.tensor_add(y_t[:], y_t[:], lnw_sb[:, widx + 1, :])


def _transpose_into(nc, psp, dst_slices, src_t, eye_sb, tag):
    """PE-transpose src_t[128, NDK*128] into dst_slices(dk) [128,128] f32r."""
    for dk in range(NDK):
        trp = psp.tile([128, 128], F32, tag=tag, bufs=2, name=f"trp_{tag}")
        nc.tensor.transpose(trp[:], src_t[:, dk * 128:(dk + 1) * 128],
                            eye_sb[:].bitcast(F32))
        if dk % 2 == 0:
            nc.vector.tensor_copy(dst_slices(dk), trp[:])
        else:
            nc.scalar.copy(out=dst_slices(dk), in_=trp[:])


def _mark(nc, phase_marks, name):
    if phase_marks is not None:
        phase_marks.append((name, nc.next_id()))


def _body(nc, tc, trivial_ln, skip_collective, phase_marks, xs_d,
          wv_d, wo_d, w1_d, w2_d, out_d,
          u_sb, eye_sb, ones128, ones1, mask_sb, maskrep, eps_t, halfpi,
          cose_t, b1_sb, b2_sb, wqk_sb, lnw_sb):
    with ExitStack() as ctx:
        # ---------- persistent SBUF ----------
        dram = ctx.enter_context(tc.tile_pool(name="dram", bufs=1,
                                              space="DRAM"))
        ag_in = dram.tile([NTT, SCOLS], F32)
        ag_out = dram.tile([NCORES, NTT, SCOLS], F32)
        x2_dram = dram.tile([NTOK, D], F32)
        y2T_dram = dram.tile([NDK, 128, NTOK], F32)

        rc_stack = ctx.enter_context(ExitStack())
        rc_pool = rc_stack.enter_context(tc.tile_pool(name="rcp", bufs=1))
        rc_ts = [rc_pool.tile([128, SCOLS], F32R, tag=f"rc{tt}",
                              name=f"rc{tt}") for tt in range(NTT)]
        qk_all = rc_pool.tile([128, NTT, 2 * H], F32, tag="qk_all")
        cos_all = rc_pool.tile([128, NTT, 2 * H], F32, tag="cos_all")
        sin_all = rc_pool.tile([128, NTT, 2 * H], F32, tag="sin_all")

        # ================= phase A+B+C =================
        with ExitStack() as pab:
            y1T_p = pab.enter_context(tc.tile_pool(name="y1T", bufs=1))
            y1T = [y1T_p.tile([128, NTOK], F32R, tag=f"y1T{dk}",
                              name=f"y1T{dk}") for dk in range(NDK)]
            work = pab.enter_context(tc.tile_pool(name="workA", bufs=3))
            wv_sb = y1T_p.tile([128, NDK, H * DH], F32R, tag="wv")

            _mark(nc, phase_marks, 'A_ln1')
            with tc.tile_pool(name="psA", bufs=1, space="PSUM") as psA:
                for tt in range(NTT):
                    x_t = work.tile([128, D], F32, tag="x_t", bufs=3)
                    nc.sync.dma_start(
                        x_t[:], xs_d[tt * 128:(tt + 1) * 128, :].bitcast(F32))
                    y_t = work.tile([128, D], F32, tag="y_t", bufs=3)
                    _layernorm(nc, work, x_t, y_t, eps_t, lnw_sb, 0)
                    _transpose_into(
                        nc, psA,
                        lambda dk, tt=tt: y1T[dk][:, tt * 128:(tt + 1) * 128],
                        y_t, eye_sb, "trA")

                _mark(nc, phase_marks, 'B1_qk')
                for tt in range(NTT):
                    qk_ps = psA.tile([128, 2 * H], F32, tag="qk", bufs=2)
                    for dk in range(NDK):
                        nc.tensor.matmul(
                            qk_ps[:], y1T[dk][:, tt * 128:(tt + 1) * 128],
                            wqk_sb[:, dk, :],
                            start=(dk == 0), stop=(dk == NDK - 1))
                    nc.any.tensor_copy(qk_all[:, tt, :], qk_ps[:])

            # batched tanh / sin / cos
            nc.scalar.activation(out=qk_all[:], in_=qk_all[:], func=AF.Tanh)
            nc.scalar.activation(out=sin_all[:], in_=qk_all[:], func=AF.Sin,
                                 scale=PI / 4)
            nc.scalar.activation(out=cos_all[:], in_=qk_all[:], func=AF.Sin,
                                 scale=PI / 4, bias=halfpi[:])

            for dk in range(NDK):
                nc.sync.dma_start(
                    wv_sb[:, dk, :],
                    wv_d[dk * 128:(dk + 1) * 128, :])
            _mark(nc, phase_marks, 'B3_V_S_C')
            psB = pab.enter_context(
                tc.tile_pool(name="psB", bufs=1, space="PSUM"))
            for tt in range(NTT):
                v_ps = psB.tile([128, H * DH], F32, tag="v", bufs=2)
                for dk in range(NDK):
                    for nh in range(2):
                        nc.tensor.matmul(
                            v_ps[:, nh * 512:(nh + 1) * 512],
                            y1T[dk][:, tt * 128:(tt + 1) * 128],
                            wv_sb[:, dk, nh * 512:(nh + 1) * 512],
                            start=(dk == 0), stop=(dk == NDK - 1))
                s_t = work.tile([128, SCOLS], F32R, tag="s_t", bufs=2)
                v3 = v_ps[:].rearrange("p (h d) -> p h d", h=H)
                nc.vector.tensor_mul(
                    s_t[:, 0:H * DH].rearrange("p (h d) -> p h d", h=H),
                    v3,
                    cos_all[:, tt, H:2 * H][:, :, None]
                    .broadcast_to([128, H, DH]))
                nc.vector.tensor_mul(
                    s_t[:, H * DH:2 * H * DH]
                    .rearrange("p (h d) -> p h d", h=H),
                    v3,
                    sin_all[:, tt, H:2 * H][:, :, None]
                    .broadcast_to([128, H, DH]))
                nc.any.tensor_copy(s_t[:, 2 * H * DH:2 * H * DH + H],
                                   cos_all[:, tt, H:2 * H])
                nc.any.tensor_copy(s_t[:, 2 * H * DH + H:SCOLS],
                                   sin_all[:, tt, H:2 * H])
                # raw causal cumsum of S (U-matmul) into resident rc;
                # row 127 = block total feeds the carry exchange
                for c0 in range(0, SCOLS, 1024):
                    cw = min(1024, SCOLS - c0)
                    cum = psB.tile([128, 1024], F32, tag="cum", bufs=2)
                    for cc in range(0, cw, 512):
                        ccw = min(512, cw - cc)
                        nc.tensor.matmul(
                            cum[:, cc:cc + ccw], u_sb[:],
                            s_t[:, c0 + cc:c0 + cc + ccw],
                            start=True, stop=True)
                    nc.scalar.copy(out=rc_ts[tt][:, c0:c0 + cw],
                                   in_=cum[:, :cw])
                    nc.sync.dma_start(
                        ag_in[tt:tt + 1, c0:c0 + cw],
                        rc_ts[tt][127:128, c0:c0 + cw].bitcast(F32))

        _mark(nc, phase_marks, 'AG')
        # ================= AllGather =================
        if skip_collective:
            nc.gpsimd.dma_start(ag_out[0], ag_in[:])
        else:
            nc.gpsimd.collective_compute(
                "AllGather", mybir.AluOpType.bypass,
                replica_groups=[list(range(NCORES))],
                ins=[ag_in.opt()], outs=[ag_out.opt()])

        # ========== phase D: attention + residual + LN2 ==========
        _mark(nc, phase_marks, 'D_attn')
        with ExitStack() as pd:
            work = pd.enter_context(tc.tile_pool(name="workD", bufs=3))
            wo_pool = pd.enter_context(tc.tile_pool(name="wop", bufs=1))
            wo_sb = wo_pool.tile([128, NDK, D], F32R, tag="wo")
            for dk in range(NDK):
                nc.sync.dma_start(
                    wo_sb[:, dk, :],
                    wo_d[dk * 128:(dk + 1) * 128, :])

            # --- D1: batched scalar cumsums + denominators ---
            den_all = wo_pool.tile([128, NTT, H], F32, tag="den_all")
            rqc_all = wo_pool.tile([128, NTT, H], F32, tag="rqc_all")
            rqs_all = wo_pool.tile([128, NTT, H], F32, tag="rqs_all")
            with tc.tile_pool(name="psDs", bufs=1, space="PSUM") as psDs:
                csc_all = psDs.tile([128, NTT, 2 * H], F32, tag="csca")
                for tt in range(NTT):
                    b, j = tt // 2, tt % 2
                    gsc = work.tile([2 * NCORES, 2 * H], F32R, tag="gsc",
                                    bufs=2)
                    nc.sync.dma_start(
                        gsc[:],
                        ag_out[:, 2 * b:2 * b + 2, 2 * H * DH:SCOLS]
                        .bitcast(F32R))
                    nc.tensor.matmul(csc_all[:, tt, :],
                                     maskrep[:, j * 128:(j + 1) * 128],
                                     gsc[:], start=True, stop=False)
                    nc.tensor.matmul(csc_all[:, tt, :], eye_sb[:],
                                     rc_ts[tt][:, 2 * H * DH:SCOLS],
                                     start=False, stop=True)
                # batched denominators + q factors
                t2 = work.tile([128, NTT, H], F32, tag="t2")
                nc.vector.tensor_mul(den_all[:],
                                     csc_all[:, :, 0:H],
                                     cos_all[:, :, 0:H])
                nc.vector.tensor_mul(t2[:],
                                     csc_all[:, :, H:2 * H],
                                     sin_all[:, :, 0:H])
                nc.vector.tensor_add(den_all[:], den_all[:], t2[:])
                nc.vector.tensor_scalar(
                    out=den_all[:], in0=den_all[:], scalar1=cose_t[:],
                    scalar2=None, op0=mybir.AluOpType.add)
                nc.vector.reciprocal(den_all[:], den_all[:])
                nc.vector.tensor_mul(rqc_all[:], den_all[:],
                                     cos_all[:, :, 0:H])
                nc.vector.tensor_mul(rqs_all[:], den_all[:],
                                     sin_all[:, :, 0:H])

            # --- D2: per-tile heads, Wo, residual, LN2 ---
            psD = pd.enter_context(
                tc.tile_pool(name="psD", bufs=1, space="PSUM"))
            x2ws = []
            for tt in range(NTT):
                b, j = tt // 2, tt % 2
                rc_t = rc_ts[tt]
                gath = work.tile([2 * NCORES, 2 * H * DH], F32R,
                                 tag="gath", bufs=2)
                nc.sync.dma_start(
                    gath[:],
                    ag_out[:, 2 * b:2 * b + 2, 0:2 * H * DH].bitcast(F32R))

                h_t = work.tile([128, H * DH], F32, tag="h_t", bufs=2)
                tmpc = work.tile([128, H * DH], F32, tag="tmpc", bufs=2)
                for half, rqa in ((0, rqc_all), (1, rqs_all)):
                    base = half * H * DH
                    dst = tmpc if half == 0 else h_t
                    for c0 in range(0, H * DH, 512):
                        cv = psD.tile([128, 512], F32, tag="cumv", bufs=2)
                        nc.tensor.matmul(
                            cv[:],
                            maskrep[:, j * 128:(j + 1) * 128],
                            gath[:, base + c0:base + c0 + 512],
                            start=True, stop=False)
                        nc.tensor.matmul(cv[:], eye_sb[:],
                                         rc_t[:, base + c0:base + c0 + 512],
                                         start=False, stop=True)
                        nc.vector.tensor_mul(
                            dst[:, c0:c0 + 512]
                            .rearrange("p (h d) -> p h d", h=H // 2),
                            cv[:].rearrange("p (h d) -> p h d", h=H // 2),
                            rqa[:, tt, c0 // DH:(c0 + 512) // DH]
                            [:, :, None].broadcast_to([128, H // 2, DH]))

                # transpose heads (summing both halves in PSUM) + Wo + x
                x_t = work.tile([128, D], F32R, tag="x_t2", bufs=2)
                nc.sync.dma_start(x_t[:], xs_d[tt * 128:(tt + 1) * 128, :])
                attn = psD.tile([128, D], F32, tag="attn", bufs=2)
                for dk in range(NDK):
                    trp = psD.tile([128, 128], F32, tag="trD", bufs=2)
                    nc.tensor.matmul(
                        trp[:], tmpc[:, dk * 128:(dk + 1) * 128],
                        eye_sb[:].bitcast(F32), is_transpose=True,
                        start=True, stop=False)
                    nc.tensor.matmul(
                        trp[:], h_t[:, dk * 128:(dk + 1) * 128],
                        eye_sb[:].bitcast(F32), is_transpose=True,
                        start=False, stop=True)
                    hT = work.tile([128, 128], F32R, tag="hT", bufs=2)
                    nc.any.tensor_copy(hT[:], trp[:])
                    for nh in range(2):
                        nc.tensor.matmul(
                            attn[:, nh * 512:(nh + 1) * 512], hT[:],
                            wo_sb[:, dk, nh * 512:(nh + 1) * 512],
                            start=(dk == 0), stop=False)
                for nh in range(2):
                    nc.tensor.matmul(
                        attn[:, nh * 512:(nh + 1) * 512], eye_sb[:],
                        x_t[:, nh * 512:(nh + 1) * 512],
                        start=False, stop=True)
                x2w = work.tile([128, D], F32, tag=f"x2w{tt}",
                                name=f"x2w{tt}", bufs=1)
                nc.scalar.copy(out=x2w[:], in_=attn[:])
                nc.sync.dma_start(x2_dram[tt * 128:(tt + 1) * 128, :],
                                  x2w[:])
                x2ws.append(x2w)

            # --- D3: LN2 + transpose y2 (separate loop so the LN chain
            # does not break the attention pipeline) ---
            for tt in range(NTT):
                y_t = work.tile([128, D], F32, tag="y2_t", bufs=2)
                _layernorm(nc, work, x2ws[tt], y_t, eps_t, lnw_sb, 2)
                y2s = work.tile([128, NDK, 128], F32R, tag="y2s", bufs=3)
                _transpose_into(
                    nc, psD,
                    lambda dk: y2s[:, dk, :],
                    y_t, eye_sb, "trD")
                nc.sync.dma_start(
                    y2T_dram[:, :, tt * 128:(tt + 1) * 128]
                    .rearrange("k p m -> p k m"),
                    y2s[:].bitcast(F32))

        rc_stack.close()

        # ================= phase E: MLP =================
        _mark(nc, phase_marks, 'E_mlp')
        with ExitStack() as pe:
            y2T_p = pe.enter_context(tc.tile_pool(name="y2Tp", bufs=1))
            y2T = [y2T_p.tile([128, NTOK], F32R, tag=f"y2T{dk}",
                              name=f"y2T{dk}") for dk in range(NDK)]
            for dk in range(NDK):
                nc.sync.dma_start(y2T[dk][:], y2T_dram[dk].bitcast(F32R))
            wpool = pe.enter_context(tc.tile_pool(name="wmlp", bufs=3))
            h1_pool = pe.enter_context(tc.tile_pool(name="h1p", bufs=1))
            opool = pe.enter_context(tc.tile_pool(name="outp", bufs=3))
            h1 = h1_pool.tile([128, NMT, NTOK], F32R, tag="h1")
            with tc.tile_pool(name="psE1", bufs=1, space="PSUM") as psE1:
                for mt in range(NMT):
                    w1_t = wpool.tile([128, NDK, 128], F32R, tag="w1t")
                    nc.sync.dma_start(
                        w1_t[:],
                        w1_d.rearrange("(k p) m -> p k m", p=128)
                        [:, :, mt * 128:(mt + 1) * 128])
                    h1ps = psE1.tile([128, NTOK], F32, tag="h1ps", bufs=2)
                    for dk in range(NDK):
                        for nh in range(2):
                            nc.tensor.matmul(
                                h1ps[:, nh * 512:(nh + 1) * 512],
                                w1_t[:, dk, :],
                                y2T[dk][:, nh * 512:(nh + 1) * 512],
                                start=(dk == 0), stop=(dk == NDK - 1))
                    nc.scalar.activation(
                        out=h1[:, mt, :], in_=h1ps[:],
                        func=AF.Gelu_apprx_tanh,
                        bias=b1_sb[:, mt:mt + 1], scale=1.0)
            _mark(nc, phase_marks, 'E2_mlp2')
            with tc.tile_pool(name="psE2", bufs=1, space="PSUM") as psE2:
                for dhalf in range(2):
                    d0 = dhalf * 512
                    ops = [psE2.tile([128, 512], F32, tag=f"o{tt}",
                                     name=f"ops{tt}") for tt in range(NTT)]
                    for mt in range(NMT):
                        w2_t = wpool.tile([128, 512], F32R, tag="w2t", bufs=4)
                        nc.sync.dma_start(
                            w2_t[:],
                            w2_d[mt * 128:(mt + 1) * 128, d0:d0 + 512])
                        for tt in range(NTT):
                            nc.tensor.matmul(
                                ops[tt][:],
                                h1[:, mt, tt * 128:(tt + 1) * 128],
                                w2_t[:],
                                start=(mt == 0), stop=False)
                    for tt in range(NTT):
                        nc.tensor.matmul(ops[tt][:], ones1[:],
                                         b2_sb[:, d0:d0 + 512],
                                         start=False, stop=True)
                        x2r = opool.tile([128, 512], F32, tag="x2r")
                        nc.sync.dma_start(
                            x2r[:],
                            x2_dram[tt * 128:(tt + 1) * 128, d0:d0 + 512])
                        o_t = opool.tile([128, 512], F32, tag="o_t")
                        nc.vector.tensor_add(o_t[:], ops[tt][:], x2r[:])
                        nc.sync.dma_start(
                            out_d[tt * 128:(tt + 1) * 128, d0:d0 + 512],
                            o_t[:])


# ---------------------------------------------------------------------------
# host side
# ---------------------------------------------------------------------------

def _prep_inputs(x, W_Q, W_K, W_V, W_O, ln1_w, ln1_b, ln2_w, ln2_b,
                 W1, b1, W2, b2):
    f = np.float32
    wqk = np.concatenate(
        [np.asarray(W_Q)[:, :, 0].T, np.asarray(W_K)[:, :, 0].T],
        axis=1).astype(f)                                        # [D, 2H]
    wv = np.ascontiguousarray(
        np.asarray(W_V).transpose(1, 0, 2).reshape(D, H * DH)).astype(f)
    wo = np.ascontiguousarray(
        np.asarray(W_O).transpose(2, 1, 0).reshape(H * DH, D)).astype(f)
    b1r = np.ascontiguousarray(np.asarray(b1, f).reshape(NMT, 128).T)
    b2r = np.asarray(b2, f).reshape(1, D)
    utri = np.triu(np.ones((128, 128), f))
    eye = np.eye(128, dtype=f)
    ones128 = np.ones((128, 1), f)
    ones1 = np.ones((1, 128), f)
    lnw = np.stack([np.asarray(ln1_w, f), np.asarray(ln1_b, f),
                    np.asarray(ln2_w, f), np.asarray(ln2_b, f)])
    common = dict(wqk=wqk, wv=wv, wo=wo,
                  w1=np.asarray(W1, f), w2=np.asarray(W2, f),
                  b1r=b1r, b2r=b2r, utri=utri, eye=eye,
                  ones128=ones128, ones1=ones1, lnw=lnw)
    x = np.asarray(x, f)
    in_maps = []
    for c in range(NCORES):
        xs = np.ascontiguousarray(
            x[:, c * TC:(c + 1) * TC, :].reshape(NTOK, D))
        masks = np.zeros((2 * NCORES, 2), f)
        for cp in range(NCORES):
            for jp in range(2):
                row = 2 * cp + jp
                masks[row, 0] = 1.0 if cp < c else 0.0
                masks[row, 1] = 1.0 if (cp < c or (cp == c and jp == 0)) \
                    else 0.0
        maskrep = np.concatenate(
            [np.repeat(masks[:, jj:jj + 1], 128, axis=1) for jj in range(2)],
            axis=1)
        in_maps.append(dict(common, xs=xs, masks=masks, maskrep=maskrep))
    trivial = bool(np.allclose(ln1_w, 1) and np.allclose(ln2_w, 1)
                   and np.allclose(ln1_b, 0) and np.allclose(ln2_b, 0))
    return in_maps, trivial


_CACHE = {}


def make_runner(nc):
    """Build a reusable jitted callable for this compiled Bass program."""
    import jax
    from jax.sharding import Mesh, PartitionSpec
    from jax.experimental.shard_map import shard_map

    bass2jax.install_neuronx_cc_hook()
    partition_name = (nc.partition_id_tensor.name
                      if nc.partition_id_tensor else None)
    in_names, out_names, out_avals, zero_outs = [], [], [], []
    for alloc in nc.m.functions[0].allocations:
        if not isinstance(alloc, mybir.MemoryLocationSet):
            continue
        name = alloc.memorylocations[0].name
        if alloc.kind == "ExternalInput":
            if name != partition_name:
                in_names.append(name)
        elif alloc.kind == "ExternalOutput":
            out_names.append(name)
            shape = tuple(alloc.tensor_shape)
            dtype = mybir.dt.np(alloc.dtype)
            out_avals.append(jax.core.ShapedArray(shape, dtype))
            zero_outs.append(np.zeros(shape, dtype))
    n_params = len(in_names)
    n_outs = len(out_avals)
    in_names_all = in_names + out_names
    if partition_name is not None:
        in_names_all.append(partition_name)

    def _bodyfn(*args):
        operands = list(args)
        if partition_name is not None:
            operands.append(bass2jax.partition_id_tensor())
        outs = bass2jax._bass_exec_p.bind(
            *operands,
            out_avals=tuple(out_avals),
            in_names=tuple(in_names_all),
            out_names=tuple(out_names),
            lowering_input_output_aliases=(),
            sim_require_finite=True,
            sim_require_nnan=True,
            nc=nc,
        )
        return tuple(outs)

    devices = jax.devices()[:NCORES]
    mesh = Mesh(np.asarray(devices), ("core",))
    sharded = jax.jit(
        shard_map(_bodyfn, mesh=mesh,
                  in_specs=(PartitionSpec("core"),) * (n_params + n_outs),
                  out_specs=(PartitionSpec("core"),) * n_outs,
                  check_rep=False),
        keep_unused=True)

    def run(in_maps):
        concat_in = [
            np.concatenate([np.asarray(m[name]) for m in in_maps], axis=0)
            for name in in_names
        ]
        zeros = [np.zeros((NCORES * z.shape[0], *z.shape[1:]), z.dtype)
                 for z in zero_outs]
        outs = sharded(*concat_in, *zeros)
        jax.block_until_ready(outs)
        return {
            name: np.asarray(outs[i]).reshape(NCORES, *out_avals[i].shape)
            for i, name in enumerate(out_names)
        }

    return run


def kernel(**inputs):
    in_maps, trivial = _prep_inputs(**inputs)
    key = ("prog", trivial)
    if key not in _CACHE:
        _CACHE[key] = build_program(trivial_ln=trivial)
    nc = _CACHE[key]
    rkey = ("run", trivial)
    if rkey not in _CACHE:
        _CACHE[rkey] = make_runner(nc)
    outs = _CACHE[rkey](in_maps)
    out = outs["out"]  # [NCORES, NTOK, D]
    res = np.empty((B, T, D), np.float32)
    for c in range(NCORES):
        res[:, c * TC:(c + 1) * TC, :] = out[c].reshape(B, TC, D)
    return res



# revision 4
# speedup vs baseline: 14481.0853x; 14481.0853x over previous
"""Trainium2 Bass kernel for nn_CosBlock (cos-attention transformer block).

Computation (B=4, T=2048, D=1024, H=16, Dh=64, Dmlp=4096), fp32:
    y  = LN1(x)
    q,k = tanh(y @ Wq|k) * pi/4 ; V = y @ Wv          (per head)
    cos-linear-attention via causal cumsum over T, normalized
    x2 = x + attn @ Wo
    out = x2 + gelu(LN2(x2) @ W1 + b1) @ W2 + b2

Distribution: tokens sharded over 8 cores (T split into 8 chunks of 256
per batch).  The only cross-core dependency is the cumsum carry: each
core AllGathers its per-128-block partial sums (tiny, 33KB/core bf16)
and adds a prefix offset computed with a per-core 0/1 mask matmul.

All heavy matmuls run in bf16 (activations rounded to bf16, fp32 PSUM
accumulation, ~1e-3 rel err).  bf16 keeps the PE fast-weight-load path
enabled (f32r disables it and roughly doubles per-matmul cost) and
halves HBM traffic.  Transposes are regular X.T@I matmuls (bf16 in,
fp32 PSUM out) rather than transpose-mode, which keeps the PE activity
monitor warm.  Cumsum is an upper-triangular matmul per 128-token block
with carry offsets folded into the same PSUM accumulation.
"""
from contextlib import ExitStack

import numpy as np

import concourse.bacc as bacc
import concourse.tile as tile
import concourse.mybir as mybir
from concourse import bass2jax

F32 = mybir.dt.float32
BF16 = mybir.dt.bfloat16
AF = mybir.ActivationFunctionType
PI = float(np.pi)
LN_EPS = 1e-5
COS_EPS = 1e-6

NCORES = 8
B, T, D, H, DH, DMLP = 4, 2048, 1024, 16, 64, 4096
TC = T // NCORES          # tokens per core per batch = 256
NTOK = B * TC             # tokens per core = 1024
NTT = NTOK // 128         # token tiles per core = 8  (= B * 2 blocks)
NDK = D // 128            # 8
NMT = DMLP // 128         # 32
SCOLS = 2 * H * DH + 2 * H  # 2080 = cos*V | sin*V | cos_k | sin_k


def build_program(trivial_ln=True, repeats=1, n_devices=NCORES,
                  skip_collective=False, phase_marks=None):
    nc = bacc.Bacc("TRN2", target_bir_lowering=False, debug=False,
                   num_devices=n_devices)

    def din(name, shape, dt=BF16):
        return nc.dram_tensor(name, shape, dt, kind="ExternalInput").ap()

    xs_d = din("xs", [NTOK, D], F32)
    wqk_d = din("wqk", [D, 2 * H])
    wv_d = din("wv", [D, H * DH])
    wo_d = din("wo", [H * DH, D])
    w1_d = din("w1p", [128, NMT, NDK * 128])
    w2_d = din("w2", [DMLP, D])
    b1_d = din("b1r", [128, NMT], F32)
    b2_d = din("b2r", [1, D])
    u_d = din("utri", [128, 128])
    eye_d = din("eye", [128, 128])
    ones1_d = din("ones1", [1, 128])
    maskrep_d = din("maskrep", [2 * NCORES, 2 * 128])
    lnw_d = din("lnw", [4, D], F32)  # ln1_w, ln1_b, ln2_w, ln2_b rows
    out_d = nc.dram_tensor("out", [NTOK, D], F32, kind="ExternalOutput").ap()

    with tile.TileContext(nc) as tc, ExitStack() as top:
        consts = top.enter_context(tc.tile_pool(name="consts", bufs=1))
        u_sb = consts.tile([128, 128], BF16)
        eye_sb = consts.tile([128, 128], BF16)
        ones1 = consts.tile([1, 128], BF16)
        maskrep = consts.tile([2 * NCORES, 2 * 128], BF16)
        eps_t = consts.tile([128, 1], F32)
        halfpi = consts.tile([128, 1], F32)
        cose_t = consts.tile([128, 1], F32)
        b1_sb = consts.tile([128, NMT], F32)
        b2_sb = consts.tile([1, D], BF16)
        wqk_sb = consts.tile([128, NDK, 2 * H], BF16)
        nc.sync.dma_start(u_sb[:], u_d)
        nc.sync.dma_start(eye_sb[:], eye_d)
        nc.sync.dma_start(wqk_sb[:],
                          wqk_d.rearrange("(k p) n -> p k n", p=128))
        # not needed until phases D/E — keep them off the sync queue so
        # the first x-tile loads go out immediately
        nc.gpsimd.dma_start(ones1[:], ones1_d)
        nc.gpsimd.dma_start(maskrep[:], maskrep_d)
        nc.gpsimd.dma_start(b1_sb[:], b1_d)
        nc.gpsimd.dma_start(b2_sb[:], b2_d)
        nc.vector.memset(eps_t[:], LN_EPS)
        nc.vector.memset(halfpi[:], PI / 2)
        nc.vector.memset(cose_t[:], COS_EPS)
        lnw_sb = None
        if not trivial_ln:
            lnw_sb = consts.tile([128, 4, D], F32)
            nc.sync.dma_start(
                lnw_sb[:], lnw_d[None, :, :].broadcast_to([128, 4, D]))

        for _rep in range(repeats):
            _body(nc, tc, trivial_ln, skip_collective, phase_marks,
                  xs_d, wv_d, wo_d, w1_d, w2_d, out_d,
                  u_sb, eye_sb, ones1, maskrep, eps_t,
                  halfpi, cose_t, b1_sb, b2_sb, wqk_sb, lnw_sb)

    nc.compile()
    return nc


def _layernorm(nc, pool, x_t, y_t, eps_t, lnw_sb, widx):
    """token-major LN: y_t[128,1024] = LN(x_t).  lnw_sb rows widx,widx+1."""
    stats = pool.tile([128, 6 * nc.vector.BN_STATS_DIM], F32, tag="ln_stats")
    nsub = D // 512
    st3 = stats[:].rearrange("p (s d) -> p s d", s=6)
    xg = x_t[:].rearrange("p (s d) -> p s d", s=nsub)
    for s in range(nsub):
        nc.vector.bn_stats(out=st3[:, s, :], in_=xg[:, s, :])
    mv = pool.tile([128, nc.vector.BN_AGGR_DIM], F32, tag="ln_mv")
    nc.vector.bn_aggr(out=mv[:], in_=stats[:, : nsub * nc.vector.BN_STATS_DIM]
                      .rearrange("p (s d) -> p s d", s=nsub))
    rstd = pool.tile([128, 1], F32, tag="ln_rstd")
    nc.scalar.activation(out=rstd[:], in_=mv[:, 1:2], func=AF.Sqrt,
                         bias=eps_t[:], scale=1.0)
    nc.vector.reciprocal(rstd[:], rstd[:])
    nc.vector.tensor_scalar(
        out=y_t[:], in0=x_t[:], scalar1=mv[:, 0:1], scalar2=rstd[:],
        op0=mybir.AluOpType.subtract, op1=mybir.AluOpType.mult)
    if lnw_sb is not None:
        nc.vector.tensor_mul(y_t[:], y_t[:], lnw_sb[:, widx, :])
        nc.vector.tensor_add(y_t[:], y_t[:], lnw_sb[:, widx + 1, :])


def _transpose_into(nc, psp, dst_slices, src_t, eye_sb, tag):
    """Transpose src_t[128, NDK*128] bf16 into dst_slices(dk) [128,128].

    Regular matmul X.T @ I (not transpose-mode): bf16 keeps FWL on and
    counts as PE activity for the HAM clock gate."""
    for dk in range(NDK):
        trp = psp.tile([128, 128], F32, tag=tag, bufs=2, name=f"trp_{tag}")
        nc.tensor.matmul(trp[:], src_t[:, dk * 128:(dk + 1) * 128],
                         eye_sb[:], start=True, stop=True)
        if dk % 2 == 0:
            nc.vector.tensor_copy(dst_slices(dk), trp[:])
        else:
            nc.scalar.copy(out=dst_slices(dk), in_=trp[:])


def _mark(nc, phase_marks, name):
    if phase_marks is not None:
        phase_marks.append((name, nc.next_id()))


class _Scope:
    """Re-enterable named-scope helper: sc('name') opens, closing previous."""

    def __init__(self, nc):
        self.nc = nc
        self.cur = None

    def __call__(self, name):
        if self.cur is not None:
            self.nc.leave_named_scope(self.cur[0], self.cur[1], notify=False)
        self.cur = None
        if name is not None:
            sid, _ = self.nc.enter_named_scope(name, notify=False)
            self.cur = (name, sid)


def _body(nc, tc, trivial_ln, skip_collective, phase_marks, xs_d,
          wv_d, wo_d, w1_d, w2_d, out_d,
          u_sb, eye_sb, ones1, maskrep, eps_t, halfpi,
          cose_t, b1_sb, b2_sb, wqk_sb, lnw_sb):
    sc = _Scope(nc)
    with ExitStack() as ctx:
        # ---------- persistent DRAM (collective bufs only) ----------
        dram = ctx.enter_context(tc.tile_pool(name="dram", bufs=1,
                                              space="DRAM"))
        ag_in = dram.tile([NTT, SCOLS], BF16)
        ag_out = dram.tile([NCORES, NTT, SCOLS], BF16)

        # persistent across D->E: x2 tiles + transposed LN2 output
        de_pool = ctx.enter_context(tc.tile_pool(name="dep", bufs=1))
        x2ws = [de_pool.tile([128, D], F32, tag=f"x2w{tt}",
                             name=f"x2w{tt}") for tt in range(NTT)]
        y2T = [de_pool.tile([128, NTOK], BF16, tag=f"y2T{dk}",
                            name=f"y2T{dk}") for dk in range(NDK)]

        rc_stack = ctx.enter_context(ExitStack())
        rc_pool = rc_stack.enter_context(tc.tile_pool(name="rcp", bufs=1))
        rc_ts = [rc_pool.tile([128, SCOLS], BF16, tag=f"rc{tt}",
                              name=f"rc{tt}") for tt in range(NTT)]
        qk_all = rc_pool.tile([128, NTT, 2 * H], F32, tag="qk_all")
        cos_all = rc_pool.tile([128, NTT, 2 * H], F32, tag="cos_all")
        sin_all = rc_pool.tile([128, NTT, 2 * H], F32, tag="sin_all")

        # ================= phase A+B+C =================
        with ExitStack() as pab:
            y1T_p = pab.enter_context(tc.tile_pool(name="y1T", bufs=1))
            y1T = [y1T_p.tile([128, NTOK], BF16, tag=f"y1T{dk}",
                              name=f"y1T{dk}") for dk in range(NDK)]
            work = pab.enter_context(tc.tile_pool(name="workA", bufs=3))
            wv_sb = y1T_p.tile([128, NDK, H * DH], BF16, tag="wv")

            _mark(nc, phase_marks, 'A_ln1')
            sc('A_ln1')
            with tc.tile_pool(name="psA", bufs=1, space="PSUM") as psA:
                for tt in range(NTT):
                    x_t = work.tile([128, D], F32, tag="x_t", bufs=3)
                    nc.sync.dma_start(
                        x_t[:], xs_d[tt * 128:(tt + 1) * 128, :])
                    y_t = work.tile([128, D], BF16, tag="y_t", bufs=3)
                    _layernorm(nc, work, x_t, y_t, eps_t, lnw_sb, 0)
                    _transpose_into(
                        nc, psA,
                        lambda dk, tt=tt: y1T[dk][:, tt * 128:(tt + 1) * 128],
                        y_t, eye_sb, "trA")

                _mark(nc, phase_marks, 'B1_qk')
                sc('B1_qk')
                for tt in range(NTT):
                    qk_ps = psA.tile([128, 2 * H], F32, tag="qk", bufs=2)
                    for dk in range(NDK):
                        nc.tensor.matmul(
                            qk_ps[:], y1T[dk][:, tt * 128:(tt + 1) * 128],
                            wqk_sb[:, dk, :],
                            start=(dk == 0), stop=(dk == NDK - 1))
                    nc.any.tensor_copy(qk_all[:, tt, :], qk_ps[:])

            # batched tanh / sin / cos
            nc.scalar.activation(out=qk_all[:], in_=qk_all[:], func=AF.Tanh)
            nc.scalar.activation(out=sin_all[:], in_=qk_all[:], func=AF.Sin,
                                 scale=PI / 4)
            nc.scalar.activation(out=cos_all[:], in_=qk_all[:], func=AF.Sin,
                                 scale=PI / 4, bias=halfpi[:])

            for dk in range(NDK):
                nc.sync.dma_start(
                    wv_sb[:, dk, :],
                    wv_d[dk * 128:(dk + 1) * 128, :])
            _mark(nc, phase_marks, 'B3_V_S_C')
            sc('B3_V_S_C')
            psB = pab.enter_context(
                tc.tile_pool(name="psB", bufs=1, space="PSUM"))
            for tt in range(NTT):
                v_ps = psB.tile([128, H * DH], F32, tag="v", bufs=2)
                for dk in range(NDK):
                    for nh in range(2):
                        nc.tensor.matmul(
                            v_ps[:, nh * 512:(nh + 1) * 512],
                            y1T[dk][:, tt * 128:(tt + 1) * 128],
                            wv_sb[:, dk, nh * 512:(nh + 1) * 512],
                            start=(dk == 0), stop=(dk == NDK - 1))
                s_t = work.tile([128, SCOLS], BF16, tag="s_t", bufs=2)
                v3 = v_ps[:].rearrange("p (h d) -> p h d", h=H)
                nc.vector.tensor_mul(
                    s_t[:, 0:H * DH].rearrange("p (h d) -> p h d", h=H),
                    v3,
                    cos_all[:, tt, H:2 * H][:, :, None]
                    .broadcast_to([128, H, DH]))
                nc.vector.tensor_mul(
                    s_t[:, H * DH:2 * H * DH]
                    .rearrange("p (h d) -> p h d", h=H),
                    v3,
                    sin_all[:, tt, H:2 * H][:, :, None]
                    .broadcast_to([128, H, DH]))
                nc.any.tensor_copy(s_t[:, 2 * H * DH:2 * H * DH + H],
                                   cos_all[:, tt, H:2 * H])
                nc.any.tensor_copy(s_t[:, 2 * H * DH + H:SCOLS],
                                   sin_all[:, tt, H:2 * H])
                # raw causal cumsum of S (U-matmul) into resident rc;
                # row 127 = block total feeds the carry exchange
                for c0 in range(0, SCOLS, 1024):
                    cw = min(1024, SCOLS - c0)
                    cum = psB.tile([128, 1024], F32, tag="cum", bufs=2)
                    for cc in range(0, cw, 512):
                        ccw = min(512, cw - cc)
                        nc.tensor.matmul(
                            cum[:, cc:cc + ccw], u_sb[:],
                            s_t[:, c0 + cc:c0 + cc + ccw],
                            start=True, stop=True)
                    nc.scalar.copy(out=rc_ts[tt][:, c0:c0 + cw],
                                   in_=cum[:, :cw])
                    nc.sync.dma_start(
                        ag_in[tt:tt + 1, c0:c0 + cw],
                        rc_ts[tt][127:128, c0:c0 + cw])

        _mark(nc, phase_marks, 'AG')
        sc('AG')
        # ================= AllGather =================
        if skip_collective:
            nc.gpsimd.dma_start(ag_out[0], ag_in[:])
        else:
            nc.gpsimd.collective_compute(
                "AllGather", mybir.AluOpType.bypass,
                replica_groups=[list(range(NCORES))],
                ins=[ag_in.opt()], outs=[ag_out.opt()])

        # ========== phase D: attention + residual + LN2 ==========
        _mark(nc, phase_marks, 'D_attn')
        sc('D_attn')
        with ExitStack() as pd:
            work = pd.enter_context(tc.tile_pool(name="workD", bufs=3))
            wo_pool = pd.enter_context(tc.tile_pool(name="wop", bufs=1))
            wo_sb = wo_pool.tile([128, NDK, D], BF16, tag="wo")
            for dk in range(NDK):
                nc.sync.dma_start(
                    wo_sb[:, dk, :],
                    wo_d[dk * 128:(dk + 1) * 128, :])

            # --- D1: batched scalar cumsums + denominators ---
            den_all = wo_pool.tile([128, NTT, H], F32, tag="den_all")
            rqc_all = wo_pool.tile([128, NTT, H], F32, tag="rqc_all")
            rqs_all = wo_pool.tile([128, NTT, H], F32, tag="rqs_all")
            with tc.tile_pool(name="psDs", bufs=1, space="PSUM") as psDs:
                csc_all = psDs.tile([128, NTT, 2 * H], F32, tag="csca")
                for tt in range(NTT):
                    b, j = tt // 2, tt % 2
                    gsc = work.tile([2 * NCORES, 2 * H], BF16, tag="gsc",
                                    bufs=2)
                    nc.sync.dma_start(
                        gsc[:],
                        ag_out[:, 2 * b:2 * b + 2, 2 * H * DH:SCOLS])
                    nc.tensor.matmul(csc_all[:, tt, :],
                                     maskrep[:, j * 128:(j + 1) * 128],
                                     gsc[:], start=True, stop=False)
                    nc.tensor.matmul(csc_all[:, tt, :], eye_sb[:],
                                     rc_ts[tt][:, 2 * H * DH:SCOLS],
                                     start=False, stop=True)
                # batched denominators + q factors
                t2 = work.tile([128, NTT, H], F32, tag="t2")
                nc.vector.tensor_mul(den_all[:],
                                     csc_all[:, :, 0:H],
                                     cos_all[:, :, 0:H])
                nc.vector.tensor_mul(t2[:],
                                     csc_all[:, :, H:2 * H],
                                     sin_all[:, :, 0:H])
                nc.vector.tensor_add(den_all[:], den_all[:], t2[:])
                nc.vector.tensor_scalar(
                    out=den_all[:], in0=den_all[:], scalar1=cose_t[:],
                    scalar2=None, op0=mybir.AluOpType.add)
                nc.vector.reciprocal(den_all[:], den_all[:])
                nc.vector.tensor_mul(rqc_all[:], den_all[:],
                                     cos_all[:, :, 0:H])
                nc.vector.tensor_mul(rqs_all[:], den_all[:],
                                     sin_all[:, :, 0:H])

            # --- D2: per-tile heads, Wo, residual ---
            psD = pd.enter_context(
                tc.tile_pool(name="psD", bufs=1, space="PSUM"))
            for tt in range(NTT):
                b, j = tt // 2, tt % 2
                rc_t = rc_ts[tt]
                gath = work.tile([2 * NCORES, 2 * H * DH], BF16,
                                 tag="gath", bufs=2)
                nc.sync.dma_start(
                    gath[:],
                    ag_out[:, 2 * b:2 * b + 2, 0:2 * H * DH])

                h_t = work.tile([128, H * DH], BF16, tag="h_t", bufs=2)
                tmpc = work.tile([128, H * DH], BF16, tag="tmpc", bufs=2)
                for half, rqa in ((0, rqc_all), (1, rqs_all)):
                    base = half * H * DH
                    dst = tmpc if half == 0 else h_t
                    for c0 in range(0, H * DH, 512):
                        cv = psD.tile([128, 512], F32, tag="cumv", bufs=2)
                        nc.tensor.matmul(
                            cv[:],
                            maskrep[:, j * 128:(j + 1) * 128],
                            gath[:, base + c0:base + c0 + 512],
                            start=True, stop=False)
                        nc.tensor.matmul(cv[:], eye_sb[:],
                                         rc_t[:, base + c0:base + c0 + 512],
                                         start=False, stop=True)
                        nc.vector.tensor_mul(
                            dst[:, c0:c0 + 512]
                            .rearrange("p (h d) -> p h d", h=H // 2),
                            cv[:].rearrange("p (h d) -> p h d", h=H // 2),
                            rqa[:, tt, c0 // DH:(c0 + 512) // DH]
                            [:, :, None].broadcast_to([128, H // 2, DH]))

                # transpose heads (summing both halves in PSUM) + Wo
                x_t = work.tile([128, D], F32, tag="x_t2", bufs=2)
                nc.sync.dma_start(x_t[:], xs_d[tt * 128:(tt + 1) * 128, :])
                attn = psD.tile([128, D], F32, tag="attn", bufs=2)
                for dk in range(NDK):
                    trp = psD.tile([128, 128], F32, tag="trD", bufs=2)
                    nc.tensor.matmul(
                        trp[:], tmpc[:, dk * 128:(dk + 1) * 128],
                        eye_sb[:], start=True, stop=False)
                    nc.tensor.matmul(
                        trp[:], h_t[:, dk * 128:(dk + 1) * 128],
                        eye_sb[:], start=False, stop=True)
                    hT = work.tile([128, 128], BF16, tag="hT", bufs=2)
                    nc.any.tensor_copy(hT[:], trp[:])
                    for nh in range(2):
                        nc.tensor.matmul(
                            attn[:, nh * 512:(nh + 1) * 512], hT[:],
                            wo_sb[:, dk, nh * 512:(nh + 1) * 512],
                            start=(dk == 0), stop=(dk == NDK - 1))
                # residual add on DVE (PSUM + SBUF -> SBUF)
                nc.vector.tensor_add(x2ws[tt][:], attn[:], x_t[:])

            # --- D3: LN2 + transpose y2 (separate loop so the LN chain
            # does not break the attention pipeline) ---
            sc('D3_ln2')
            for tt in range(NTT):
                y_t = work.tile([128, D], BF16, tag="y2_t", bufs=2)
                _layernorm(nc, work, x2ws[tt], y_t, eps_t, lnw_sb, 2)
                _transpose_into(
                    nc, psD,
                    lambda dk, tt=tt: y2T[dk][:, tt * 128:(tt + 1) * 128],
                    y_t, eye_sb, "trD")

        rc_stack.close()

        # ================= phase E: MLP =================
        _mark(nc, phase_marks, 'E_mlp')
        sc('E_mlp')
        with ExitStack() as pe:
            wpool = pe.enter_context(tc.tile_pool(name="wmlp", bufs=3))
            h1_pool = pe.enter_context(tc.tile_pool(name="h1p", bufs=1))
            opool = pe.enter_context(tc.tile_pool(name="outp", bufs=3))
            h1 = h1_pool.tile([128, NMT, NTOK], BF16, tag="h1")
            with tc.tile_pool(name="psE1", bufs=1, space="PSUM") as psE1:
                for mt in range(NMT):
                    w1_t = wpool.tile([128, NDK, 128], BF16, tag="w1t")
                    nc.sync.dma_start(w1_t[:], w1_d[:, mt])
                    h1ps = psE1.tile([128, NTOK], F32, tag="h1ps", bufs=2)
                    for dk in range(NDK):
                        for nh in range(2):
                            nc.tensor.matmul(
                                h1ps[:, nh * 512:(nh + 1) * 512],
                                w1_t[:, dk, :],
                                y2T[dk][:, nh * 512:(nh + 1) * 512],
                                start=(dk == 0), stop=(dk == NDK - 1))
                    nc.scalar.activation(
                        out=h1[:, mt, :], in_=h1ps[:],
                        func=AF.Gelu_apprx_tanh,
                        bias=b1_sb[:, mt:mt + 1], scale=1.0)
            _mark(nc, phase_marks, 'E2_mlp2')
            sc('E2_mlp2')
            with tc.tile_pool(name="psE2", bufs=1, space="PSUM") as psE2:
                for dhalf in range(2):
                    d0 = dhalf * 512
                    ops = [psE2.tile([128, 512], F32, tag=f"o{tt}",
                                     name=f"ops{tt}") for tt in range(NTT)]
                    for mt in range(NMT):
                        w2_t = wpool.tile([128, 512], BF16, tag="w2t", bufs=4)
                        nc.sync.dma_start(
                            w2_t[:],
                            w2_d[mt * 128:(mt + 1) * 128, d0:d0 + 512])
                        for tt in range(NTT):
                            nc.tensor.matmul(
                                ops[tt][:],
                                h1[:, mt, tt * 128:(tt + 1) * 128],
                                w2_t[:],
                                start=(mt == 0), stop=False)
                    for tt in range(NTT):
                        nc.tensor.matmul(ops[tt][:], ones1[:],
                                         b2_sb[:, d0:d0 + 512],
                                         start=False, stop=True)
                        o_t = opool.tile([128, 512], F32, tag="o_t")
                        nc.vector.tensor_add(o_t[:], ops[tt][:],
                                             x2ws[tt][:, d0:d0 + 512])
                        nc.sync.dma_start(
                            out_d[tt * 128:(tt + 1) * 128, d0:d0 + 512],
                            o_t[:])
        sc(None)


# ---------------------------------------------------------------------------
# host side
# ---------------------------------------------------------------------------

def _prep_inputs(x, W_Q, W_K, W_V, W_O, ln1_w, ln1_b, ln2_w, ln2_b,
                 W1, b1, W2, b2):
    import ml_dtypes
    f = np.float32
    bf = ml_dtypes.bfloat16
    wqk = np.concatenate(
        [np.asarray(W_Q)[:, :, 0].T, np.asarray(W_K)[:, :, 0].T],
        axis=1).astype(bf)                                       # [D, 2H]
    wv = np.ascontiguousarray(
        np.asarray(W_V).transpose(1, 0, 2).reshape(D, H * DH)).astype(bf)
    wo = np.ascontiguousarray(
        np.asarray(W_O).transpose(2, 1, 0).reshape(H * DH, D)).astype(bf)
    # w1 pre-tiled: w1p[p, mt, dk*128+c] = W1[dk*128+p, mt*128+c]
    w1p = np.ascontiguousarray(
        np.asarray(W1, f).reshape(NDK, 128, NMT, 128)
        .transpose(1, 2, 0, 3).reshape(128, NMT, NDK * 128)).astype(bf)
    b1r = np.ascontiguousarray(np.asarray(b1, f).reshape(NMT, 128).T)
    b2r = np.asarray(b2, f).reshape(1, D).astype(bf)
    utri = np.triu(np.ones((128, 128), f)).astype(bf)
    eye = np.eye(128, dtype=f).astype(bf)
    ones1 = np.ones((1, 128), bf)
    lnw = np.stack([np.asarray(ln1_w, f), np.asarray(ln1_b, f),
                    np.asarray(ln2_w, f), np.asarray(ln2_b, f)])
    common = dict(wqk=wqk, wv=wv, wo=wo,
                  w1p=w1p, w2=np.asarray(W2, f).astype(bf),
                  b1r=b1r, b2r=b2r, utri=utri, eye=eye,
                  ones1=ones1, lnw=lnw)
    x = np.asarray(x, f)
    in_maps = []
    for c in range(NCORES):
        xs = np.ascontiguousarray(
            x[:, c * TC:(c + 1) * TC, :].reshape(NTOK, D))
        masks = np.zeros((2 * NCORES, 2), f)
        for cp in range(NCORES):
            for jp in range(2):
                row = 2 * cp + jp
                masks[row, 0] = 1.0 if cp < c else 0.0
                masks[row, 1] = 1.0 if (cp < c or (cp == c and jp == 0)) \
                    else 0.0
        maskrep = np.concatenate(
            [np.repeat(masks[:, jj:jj + 1], 128, axis=1) for jj in range(2)],
            axis=1).astype(bf)
        in_maps.append(dict(common, xs=xs, maskrep=maskrep))
    trivial = bool(np.allclose(ln1_w, 1) and np.allclose(ln2_w, 1)
                   and np.allclose(ln1_b, 0) and np.allclose(ln2_b, 0))
    return in_maps, trivial


_CACHE = {}


def make_runner(nc):
    """Build a reusable jitted callable for this compiled Bass program."""
    import jax
    from jax.sharding import Mesh, PartitionSpec
    from jax.experimental.shard_map import shard_map

    bass2jax.install_neuronx_cc_hook()
    partition_name = (nc.partition_id_tensor.name
                      if nc.partition_id_tensor else None)
    in_names, out_names, out_avals, zero_outs = [], [], [], []
    for alloc in nc.m.functions[0].allocations:
        if not isinstance(alloc, mybir.MemoryLocationSet):
            continue
        name = alloc.memorylocations[0].name
        if alloc.kind == "ExternalInput":
            if name != partition_name:
                in_names.append(name)
        elif alloc.kind == "ExternalOutput":
            out_names.append(name)
            shape = tuple(alloc.tensor_shape)
            dtype = mybir.dt.np(alloc.dtype)
            out_avals.append(jax.core.ShapedArray(shape, dtype))
            zero_outs.append(np.zeros(shape, dtype))
    n_params = len(in_names)
    n_outs = len(out_avals)
    in_names_all = in_names + out_names
    if partition_name is not None:
        in_names_all.append(partition_name)

    def _bodyfn(*args):
        operands = list(args)
        if partition_name is not None:
            operands.append(bass2jax.partition_id_tensor())
        outs = bass2jax._bass_exec_p.bind(
            *operands,
            out_avals=tuple(out_avals),
            in_names=tuple(in_names_all),
            out_names=tuple(out_names),
            lowering_input_output_aliases=(),
            sim_require_finite=True,
            sim_require_nnan=True,
            nc=nc,
        )
        return tuple(outs)

    devices = jax.devices()[:NCORES]
    mesh = Mesh(np.asarray(devices), ("core",))
    sharded = jax.jit(
        shard_map(_bodyfn, mesh=mesh,
                  in_specs=(PartitionSpec("core"),) * (n_params + n_outs),
                  out_specs=(PartitionSpec("core"),) * n_outs,
                  check_rep=False),
        keep_unused=True)

    def run(in_maps):
        concat_in = [
            np.concatenate([np.asarray(m[name]) for m in in_maps], axis=0)
            for name in in_names
        ]
        zeros = [np.zeros((NCORES * z.shape[0], *z.shape[1:]), z.dtype)
                 for z in zero_outs]
        outs = sharded(*concat_in, *zeros)
        jax.block_until_ready(outs)
        return {
            name: np.asarray(outs[i]).reshape(NCORES, *out_avals[i].shape)
            for i, name in enumerate(out_names)
        }

    return run


def kernel(**inputs):
    in_maps, trivial = _prep_inputs(**inputs)
    key = ("prog", trivial)
    if key not in _CACHE:
        _CACHE[key] = build_program(trivial_ln=trivial)
    nc = _CACHE[key]
    rkey = ("run", trivial)
    if rkey not in _CACHE:
        _CACHE[rkey] = make_runner(nc)
    outs = _CACHE[rkey](in_maps)
    out = outs["out"]  # [NCORES, NTOK, D]
    res = np.empty((B, T, D), np.float32)
    for c in range(NCORES):
        res[:, c * TC:(c + 1) * TC, :] = out[c].reshape(B, TC, D)
    return res
